# revision 1
# baseline (speedup 1.0000x reference)
"""Trainium2 raw-Bass kernel for nn_DualAttentionModule (dual attention: position + channel).

Reference (B=2, C=128, H=W=64, HW=4096):
  pos  = h1x1(x) @ softmax(f1x1(x)^T g1x1(x), rows)^T + x
  chan = x @ softmax(x^T x, rows) + x          (per batch, x as (C, HW))
  y    = W1 @ pos + W2 @ chan + out_b          (out_w = [W1 | W2])

Sharding: 8 cores = 2 batches x 4 query-quarters (NQ=1024 queries each); all
cores run one SPMD program on different slices:
  - channel attention in natural [i, j] layout; exp bias = -diag(x^T x) row
    offset (safe: self-term guarantees rowsum >= 1); full P kept in SBUF;
    AV is a PSUM K-loop over i-tiles; emits full-width partial W2@chanhat.
  - position attention in transposed [j, i] layout: Pt = exp(K^T Q - 90);
    column sums via an extra ones-column matmul pass; AV accumulated in
    PSUM over j-tiles; emits the disjoint slab W1@poshat + (W1+W2)@x + bias.
Host does only input slicing/transposes, weight algebra, and the final
concat + partial-sum combine.

Written in raw Bass (explicit semaphores): the walrus build here rejects
instructions carrying more than one sync-wait, which Tile-generated sync
requires; raw streams use standalone wait_ge instructions instead.
"""

import numpy as np

C = 128
HW = 4096
NQ = 1024            # queries per core
NIT = NQ // 128      # 8 i-tiles per core
NJT = HW // 128      # 32 j-tiles
POS_OFF = 90.0       # constant exp offset for position logits (max logit ~103)

_CACHE = {}


def _build_bass(repeat=1):
    from contextlib import ExitStack

    import concourse.bass as bass
    import concourse.mybir as mybir

    f32 = mybir.dt.float32
    f32r = mybir.dt.float32r
    Exp = mybir.ActivationFunctionType.Exp
    add = mybir.AluOpType.add
    mult = mybir.AluOpType.mult
    X = mybir.AxisListType.X

    nc = bass.Bass(dynamic_dma_scratch_size=8192)

    # ---- DRAM params ----
    xc_d = nc.declare_dram_parameter("xc", [C, HW], f32, isOutput=False)
    xq_d = nc.declare_dram_parameter("xq", [C, NQ], f32, isOutput=False)
    xt_d = nc.declare_dram_parameter("xt", [NQ, C], f32, isOutput=False)
    fwT_d = nc.declare_dram_parameter("fwT", [C, C], f32, isOutput=False)
    gwT_d = nc.declare_dram_parameter("gwT", [C, C], f32, isOutput=False)
    wvpT_d = nc.declare_dram_parameter("wvpT", [C, C], f32, isOutput=False)
    w2T_d = nc.declare_dram_parameter("w2T", [C, C], f32, isOutput=False)
    w12T_d = nc.declare_dram_parameter("w12T", [C, C], f32, isOutput=False)
    fb_d = nc.declare_dram_parameter("fb", [C, 1], f32, isOutput=False)
    gb_d = nc.declare_dram_parameter("gb", [C, 1], f32, isOutput=False)
    bslab_d = nc.declare_dram_parameter("bslab", [C, 1], f32, isOutput=False)
    ones_d = nc.declare_dram_parameter("ones_c", [128, 1], f32, isOutput=False)
    onesr_d = nc.declare_dram_parameter("ones_r", [1, 128], f32, isOutput=False)
    out_slab_d = nc.declare_dram_parameter("out_slab", [C, NQ], f32, isOutput=True)
    out_chan_d = nc.declare_dram_parameter("out_chan", [C, HW], f32, isOutput=True)

    # ---- SBUF map (hand-drawn; bytes 0..8192 are pinned DMA scratch) ----
    off = [8192]

    def at(name, shape, dtype):
        h = nc.alloc_sbuf_tensor_at(name, shape, dtype, offset=off[0])
        sz = int(np.prod(shape[1:])) * 4
        off[0] += (sz + 31) // 32 * 32
        return h[:]

    xc = at("xc_sb", [C, HW], f32r)                 # 16K
    xq = at("xq_sb", [C, NQ], f32r)                 # 4K
    xt = at("xt_sb", [128, NIT, C], f32)            # 4K
    ksb = at("ksb", [C, HW], f32r)                  # 16K
    qsb = at("qsb", [C, NQ], f32r)                  # 4K
    vpt = at("vpt", [128, NJT, C], f32r)            # 16K
    chanacc = at("chanacc", [C, HW], f32r)          # 16K
    p_base = off[0]
    P = at("P_sb", [128, NIT, HW], f32r)            # 128K  (region reused below)
    after_p = off[0]
    # --- aliases inside P's region (used only after P's last read) ---
    off[0] = p_base
    ptb = at("ptb", [128, 4, NQ], f32r)             # 16K
    slab = at("slab", [C, NQ], f32)                 # 4K
    rrsb = at("rrsb", [128, NQ], f32)               # 4K
    rrow = at("rrow", [1, NQ], f32)                 # 4K
    rrec_f = at("rrec_f", [1, NQ], f32)             # 4K
    ob8 = at("ob8", [128, 8, 512], f32)             # 16K (written after P's last read)
    junk = at("junk", [128, NIT, C], f32)           # 4K (write-only)
    rrec = at("rrec", [1, NQ], f32r)                # 4K row (replicated via K=1 matmul)
    assert off[0] <= after_p
    off[0] = after_p
    # --- small persistent tensors ---
    fwT = at("fwT_sb", [C, C], f32r)
    gwT = at("gwT_sb", [C, C], f32r)
    wvpT = at("wvpT_sb", [C, C], f32r)
    w2T = at("w2T_sb", [C, C], f32r)
    w12T = at("w12T_sb", [C, C], f32r)
    ones_col = at("ones_col", [128, 1], f32r)
    onesr = at("onesr", [1, 128], f32r)
    negoff = at("negoff", [128, 1], f32)
    fb = at("fb_sb", [C, 1], f32)
    gb = at("gb_sb", [C, 1], f32)
    bslab = at("bslab_sb", [C, 1], f32)
    mi_neg = at("mi_neg", [128, NIT], f32)
    rs4 = at("rs4", [128, NIT, 4], f32)
    rc = at("rc", [128, NIT], f32)
    rcr = at("rcr", [128, NIT], f32)
    xnt = at("xnt", [128, NIT, C], f32r)            # 4K
    racc = at("racc", [128, NQ], f32r)              # 4K (pos row-sum accumulator)
    obs = [ob8[:, j] for j in range(8)]
    assert off[0] <= nc.SBUF_PARTITION_SIZE_BYTES, off[0]

    def flat(ap):
        return ap.rearrange("p a b -> p (a b)")

    # ---- schedule bookkeeping ----
    pe_seq = []
    pe_seq += [("qk", k) for k in range(32)]
    pe_seq += [("kconv", n) for n in range(8)]
    pe_seq += [("qconv", n) for n in range(2)]
    pe_seq += [("vpt", j) for j in range(NJT)]
    pe_seq += [("avc", j) for j in range(8)]
    pe_seq += [("w2", j) for j in range(8)]
    pe_seq += [("lt", 0), ("lt", 1)]
    for jt in range(NJT):
        pe_seq += [("av", jt)]
        if jt + 2 < NJT:
            pe_seq += [("lt", jt + 2)]
    pe_seq += [("rred", 0), ("rrep", 0), ("psw", 0)]
    p_val = {key: i + 1 for i, key in enumerate(pe_seq)}

    dve_seq = []
    dve_seq += [("ms", 0)]
    dve_seq += [("mi", t) for t in range(2)]
    dve_seq += [("kcopy", n) for n in range(8)]
    dve_seq += [("qcopy", n) for n in range(2)]
    dve_seq += [("vcopy", j) for j in range(NJT)]
    dve_seq += [("red", 0), ("recip", 0)]
    dve_seq += [("xnt", t) for t in range(NIT)]
    dve_seq += [("ccopy", j) for j in range(8)]
    dve_seq += [("ob", j) for j in range(8)]
    dve_seq += [("racc", j) for j in range(NJT)]
    dve_seq += [("rrow", 0), ("recf", 0), ("rrec", 0)]
    dve_seq += [("rrsb", 0), ("smul", 0), ("sadd", 0), ("sbias", 0)]
    v_val = {key: i + 1 for i, key in enumerate(dve_seq)}

    P_TOT = len(pe_seq)
    V_TOT = len(dve_seq)
    A_TOT = 64
    O_TOT = 9 * 16  # output-DMA sem per repeat

    def a_cexp(g):  # ACT counter after chan exp g completes
        return g + 1

    def a_pexp(jt):  # ACT counter after pos exp jt completes
        return 33 + jt

    ND = 11  # input DMAs on SD (xc/xq ride SD2)

    with ExitStack() as ctx:
        B01 = ctx.enter_context(nc.psum_tensor("B01", [128, 2, 512], f32))[:]
        B23 = ctx.enter_context(nc.psum_tensor("B23", [128, 2, 512], f32))[:]
        B45 = ctx.enter_context(nc.psum_tensor("B45", [128, 2, 512], f32))[:]
        B67 = ctx.enter_context(nc.psum_tensor("B67", [128, 2, 512], f32))[:]
        LQ = [B01, B23]
        SD = ctx.enter_context(nc.semaphore("SD"))
        SD2 = ctx.enter_context(nc.semaphore("SD2"))
        SP_ = ctx.enter_context(nc.semaphore("SPE"))
        SA = ctx.enter_context(nc.semaphore("SA"))
        SV = ctx.enter_context(nc.semaphore("SV"))
        SO = ctx.enter_context(nc.semaphore("SO"))
        block = ctx.enter_context(nc.Block())

        class W:
            """emit wait_ge only when the needed value exceeds what's observed"""

            def __init__(self, eng):
                self.eng = eng
                self.seen = {}

            def need(self, sem, val):
                if val > self.seen.get(id(sem), -1):
                    self.eng.wait_ge(sem, val)
                    self.seen[id(sem)] = val

        @block.sync
        def _(sync):
            w = W(sync)
            for dram, sb in ((xc_d, xc), (xq_d, xq)):
                sync.dma_start(out=sb, in_=dram[:].bitcast(f32r)).then_inc(SD2, 16)
            for dram, sb in (
                (fwT_d, fwT), (gwT_d, gwT),
                (wvpT_d, wvpT), (w2T_d, w2T), (w12T_d, w12T),
            ):
                sync.dma_start(out=sb, in_=dram[:].bitcast(f32r)).then_inc(SD, 16)
            sync.dma_start(
                out=xt, in_=xt_d[:].rearrange("(t p) c -> p t c", p=128)
            ).then_inc(SD, 16)
            for dram, sb in ((fb_d, fb), (gb_d, gb), (bslab_d, bslab)):
                sync.dma_start(out=sb, in_=dram[:]).then_inc(SD, 16)
            sync.dma_start(out=ones_col, in_=ones_d[:].bitcast(f32r)).then_inc(SD, 16)
            sync.dma_start(out=onesr, in_=onesr_d[:].bitcast(f32r)).then_inc(SD, 16)
            for r in range(repeat):
                vv = r * (V_TOT - 1)
                for jc in range(8):
                    w.need(SV, vv + v_val[("ob", jc)])
                    sync.dma_start(
                        out=out_chan_d[:, jc * 512 : (jc + 1) * 512], in_=obs[jc]
                    ).then_inc(SO, 16)
                w.need(SV, vv + v_val[("sbias", 0)])
                sync.dma_start(out=out_slab_d[:], in_=slab).then_inc(SO, 16)

        @block.tensor
        def _(pe):
            w = W(pe)
            w.need(SD2, 32)
            for r in range(repeat):
              if True:
                pp = r * P_TOT
                vv = r * (V_TOT - 1)
                aa = r * A_TOT
                if r > 0:
                    w.need(SA, aa)
                    w.need(SV, vv)
                # chan QK: group k=(it, jc4) into LQ bank pair k%2
                for k in range(32):
                    it, jc4 = k // 4, k % 4
                    if k >= 2:
                        w.need(SA, aa + a_cexp(k - 2))
                    bp = LQ[k % 2]
                    for h in range(2):
                        j0 = jc4 * 1024 + h * 512
                        m = nc.tensor.matmul(
                            bp[:, h, :],
                            xq[:, it * 128 : (it + 1) * 128],
                            xc[:, j0 : j0 + 512],
                            start=True, stop=True,
                        )
                    m.then_inc(SP_, 1)
                # pos convs fill the chan-exp shadow: K/Q via B45, Vpt via B67
                w.need(SD, 16 * ND)
                for n in range(8):
                    if n >= 2:
                        w.need(SV, vv + v_val[("kcopy", n - 2)])
                    m = nc.tensor.matmul(
                        B45[:, n % 2, :], gwT, xc[:, n * 512 : (n + 1) * 512],
                        start=True, stop=True,
                    )
                    m.then_inc(SP_, 1)
                for n in range(2):
                    w.need(SV, vv + v_val[("kcopy", 6 + n)])
                    m = nc.tensor.matmul(
                        B45[:, n % 2, :], fwT, xq[:, n * 512 : (n + 1) * 512],
                        start=True, stop=True,
                    )
                    m.then_inc(SP_, 1)
                for jt in range(NJT):
                    if jt >= 2:
                        w.need(SV, vv + v_val[("vcopy", jt - 2)])
                    m = nc.tensor.matmul(
                        B67[:, jt % 2, 0:128],
                        xc[:, jt * 128 : (jt + 1) * 128], wvpT,
                        start=True, stop=True,
                    )
                    m.then_inc(SP_, 1)
                # chan AV: accumulate over i-tiles per 512-wide chunk
                for jc in range(8):
                    w.need(SV, vv + v_val[("xnt", NIT - 1)])
                    if jc >= 2:
                        w.need(SV, vv + v_val[("ccopy", jc - 2)])
                    for it in range(NIT):
                        m = nc.tensor.matmul(
                            B45[:, jc % 2, :],
                            xnt[:, it],
                            P[:, it, jc * 512 : (jc + 1) * 512],
                            start=(it == 0), stop=(it == NIT - 1),
                        )
                    m.then_inc(SP_, 1)
                # W2 @ chanacc
                for jc in range(8):
                    w.need(SV, vv + v_val[("ccopy", jc)])
                    if jc >= 2:
                        w.need(SV, vv + v_val[("ob", jc - 2)])
                    m = nc.tensor.matmul(
                        B67[:, jc % 2, :], w2T, chanacc[:, jc * 512 : (jc + 1) * 512],
                        start=True, stop=True,
                    )
                    m.then_inc(SP_, 1)

                # pos main loop (software-pipelined: Lt two ahead of AV)
                def emit_lt(jt):
                    if jt < 2:
                        w.need(SA, aa + a_cexp(30 + jt))
                    else:
                        w.need(SA, aa + a_pexp(jt - 2))
                    w.need(SV, vv + v_val[("qcopy", 1)])
                    bp = LQ[jt % 2]
                    for h in range(2):
                        m = nc.tensor.matmul(
                            bp[:, h, :],
                            ksb[:, jt * 128 : (jt + 1) * 128],
                            qsb[:, h * 512 : (h + 1) * 512],
                            start=True, stop=True,
                        )
                    m.then_inc(SP_, 1)

                def emit_av(jt):
                    w.need(SA, aa + a_pexp(jt))
                    if jt == 0:
                        w.need(SV, vv + v_val[("ob", 7)])
                    pt = ptb[:, jt % 4]
                    for h in range(2):
                        m = nc.tensor.matmul(
                            B45[:, h, :], vpt[:, jt], pt[:, h * 512 : (h + 1) * 512],
                            start=(jt == 0), stop=(jt == NJT - 1),
                        )
                    m.then_inc(SP_, 1)

                emit_lt(0)
                emit_lt(1)
                for jt in range(NJT):
                    emit_av(jt)
                    if jt + 2 < NJT:
                        emit_lt(jt + 2)
                # tail: reduce racc over partitions; replicate 1/rsum; W12@xq
                w.need(SV, vv + v_val[("racc", NJT - 1)])
                for h in range(2):
                    m = nc.tensor.matmul(
                        B67[0:1, h, :], ones_col, racc[:, h * 512 : (h + 1) * 512],
                        start=True, stop=True,
                    )
                m.then_inc(SP_, 1)
                w.need(SA, aa + a_pexp(30))
                w.need(SV, vv + v_val[("rrec", 0)])
                for h in range(2):
                    m = nc.tensor.matmul(
                        B01[:, h, :], onesr, rrec[:, h * 512 : (h + 1) * 512],
                        start=True, stop=True,
                    )
                m.then_inc(SP_, 1)
                w.need(SA, aa + a_pexp(31))
                for h in range(2):
                    m = nc.tensor.matmul(
                        B23[:, h, :], w12T, xq[:, h * 512 : (h + 1) * 512],
                        start=True, stop=True,
                    )
                m.then_inc(SP_, 1)

        @block.scalar
        def _(act):
            w = W(act)
            for r in range(repeat):
              if True:
                pp = r * P_TOT
                vv = r * (V_TOT - 1)
                for g in range(32):
                    it, jc4 = g // 4, g % 4
                    if g == 0:
                        w.need(SV, vv + v_val[("mi", 1)])
                    w.need(SP_, pp + p_val[("qk", g)])
                    nc.scalar.activation(
                        P[:, it, jc4 * 1024 : (jc4 + 1) * 1024],
                        flat(LQ[g % 2]),
                        Exp,
                        bias=mi_neg[:, it : it + 1],
                        accum_out=rs4[:, it, jc4 : jc4 + 1],
                    ).then_inc(SA, 1)
                for jt in range(NJT):
                    w.need(SP_, pp + p_val[("lt", jt)])
                    if jt >= 4:
                        w.need(SP_, pp + p_val[("av", jt - 4)])
                        w.need(SV, vv + v_val[("racc", jt - 4)])
                    nc.scalar.activation(
                        ptb[:, jt % 4], flat(LQ[jt % 2]), Exp, bias=negoff
                    ).then_inc(SA, 1)

        @block.vector
        def _(dve):
            w = W(dve)
            nc.vector.memset(negoff, -POS_OFF).then_inc(SV, 1)
            w.need(SD, 16 * ND)
            for r in range(repeat):
              if True:
                pp = r * P_TOT
                vv = r * (V_TOT - 1)
                if r > 0:
                    w.need(SO, r * O_TOT)
                nc.vector.tensor_tensor(
                    out=flat(junk), in0=flat(xt), in1=flat(xt), op=mult
                ).then_inc(SV, 1)
                w.need(SV, vv + v_val[("mi", 0)])
                nc.vector.tensor_reduce(
                    out=mi_neg, in_=junk, axis=X, op=add, negate=True
                ).then_inc(SV, 1)
                for n in range(8):
                    w.need(SP_, pp + p_val[("kconv", n)])
                    nc.vector.tensor_scalar_add(
                        ksb[:, n * 512 : (n + 1) * 512], B45[:, n % 2, :], gb
                    ).then_inc(SV, 1)
                for n in range(2):
                    w.need(SP_, pp + p_val[("qconv", n)])
                    nc.vector.tensor_scalar_add(
                        qsb[:, n * 512 : (n + 1) * 512], B45[:, n % 2, :], fb
                    ).then_inc(SV, 1)
                for jt in range(NJT):
                    w.need(SP_, pp + p_val[("vpt", jt)])
                    nc.vector.tensor_copy(
                        vpt[:, jt], B67[:, jt % 2, 0:128]
                    ).then_inc(SV, 1)
                # chan normalization
                w.need(SA, r * A_TOT + a_cexp(31))
                nc.vector.tensor_reduce(
                    out=rc, in_=rs4, axis=X, op=add
                ).then_inc(SV, 1)
                w.need(SV, vv + v_val[("red", 0)])
                nc.vector.reciprocal(out=rcr, in_=rc).then_inc(SV, 1)
                w.need(SV, vv + v_val[("recip", 0)])
                for t in range(NIT):
                    nc.vector.tensor_scalar_mul(
                        xnt[:, t], xt[:, t], rcr[:, t : t + 1]
                    ).then_inc(SV, 1)
                for jc in range(8):
                    w.need(SP_, pp + p_val[("avc", jc)])
                    nc.vector.tensor_copy(
                        chanacc[:, jc * 512 : (jc + 1) * 512], B45[:, jc % 2, :]
                    ).then_inc(SV, 1)
                for jc in range(8):
                    w.need(SP_, pp + p_val[("w2", jc)])
                    nc.vector.tensor_copy(obs[jc], B67[:, jc % 2, :]).then_inc(SV, 1)
                # pos row-sum accumulation (replaces PE ones-matmul pass)
                for jt in range(NJT):
                    w.need(SA, r * A_TOT + a_pexp(jt))
                    if jt == 0:
                        nc.vector.tensor_copy(racc, ptb[:, 0]).then_inc(SV, 1)
                    else:
                        w.need(SV, vv + v_val[("racc", jt - 1)])
                        nc.vector.tensor_add(
                            out=racc, in0=racc, in1=ptb[:, jt % 4]
                        ).then_inc(SV, 1)
                # pos tail
                w.need(SP_, pp + p_val[("rred", 0)])
                nc.vector.tensor_copy(rrow, flat(B67[0:1])).then_inc(SV, 1)
                w.need(SV, vv + v_val[("rrow", 0)])
                nc.vector.reciprocal(out=rrec_f, in_=rrow).then_inc(SV, 1)
                w.need(SV, vv + v_val[("recf", 0)])
                nc.vector.tensor_copy(rrec, rrec_f).then_inc(SV, 1)
                w.need(SP_, pp + p_val[("rrep", 0)])
                nc.vector.tensor_copy(rrsb, flat(B01)).then_inc(SV, 1)
                w.need(SV, vv + v_val[("rrsb", 0)])
                nc.vector.tensor_mul(out=slab, in0=flat(B45), in1=rrsb).then_inc(SV, 1)
                w.need(SP_, pp + p_val[("psw", 0)])
                w.need(SV, vv + v_val[("smul", 0)])
                nc.vector.tensor_add(out=slab, in0=slab, in1=flat(B23)).then_inc(SV, 1)
                w.need(SV, vv + v_val[("sadd", 0)])
                nc.vector.tensor_scalar_add(slab, slab, bslab).then_inc(SV, 1)

    return nc


def _prep_inputs(x, f_w, f_b, g_w, g_b, h_w, h_b, out_w, out_b):
    f32 = np.float32
    x = np.ascontiguousarray(np.asarray(x, dtype=f32))
    B = x.shape[0]
    x2 = x.reshape(B, C, HW)
    W1 = np.asarray(out_w, f32)[:, :C]
    W2 = np.asarray(out_w, f32)[:, C:]
    shared = {
        "fwT": np.ascontiguousarray(np.asarray(f_w, f32).T),
        "gwT": np.ascontiguousarray(np.asarray(g_w, f32).T),
        "wvpT": np.ascontiguousarray((W1 @ np.asarray(h_w, f32)).T),
        "w2T": np.ascontiguousarray(W2.T),
        "w12T": np.ascontiguousarray((W1 + W2).T),
        "fb": np.asarray(f_b, f32).reshape(C, 1).copy(),
        "gb": np.asarray(g_b, f32).reshape(C, 1).copy(),
        "bslab": (W1 @ np.asarray(h_b, f32) + np.asarray(out_b, f32)).reshape(C, 1).copy(),
        "ones_c": np.ones((128, 1), f32),
        "ones_r": np.ones((1, 128), f32),
    }
    in_maps = []
    for core in range(8):
        b, q = core // 4, core % 4
        xqv = np.ascontiguousarray(x2[b][:, q * NQ : (q + 1) * NQ])
        in_maps.append({
            "xc": np.ascontiguousarray(x2[b]),
            "xq": xqv,
            "xt": np.ascontiguousarray(xqv.T),
            **shared,
        })
    return in_maps


def _combine(results, B):
    y = np.zeros((B, C, HW), np.float32)
    for core in range(8):
        b, q = core // 4, core % 4
        y[b, :, q * NQ : (q + 1) * NQ] += results[core]["out_slab"]
        y[b] += results[core]["out_chan"]
    return y.reshape(B, C, 64, 64)


def run_on_hw(in_maps, trace=False):
    from concourse.bass_utils import run_bass_kernel_spmd

    if "nc" not in _CACHE:
        _CACHE["nc"] = _build_bass()
    return run_bass_kernel_spmd(_CACHE["nc"], in_maps, list(range(8)), trace=trace)


def kernel(x, f_w, f_b, g_w, g_b, h_w, h_b, out_w, out_b):
    in_maps = _prep_inputs(x, f_w, f_b, g_w, g_b, h_w, h_b, out_w, out_b)
    res = run_on_hw(in_maps)
    return _combine(res.results, np.asarray(x).shape[0])

